# revision 1
# baseline (speedup 1.0000x reference)
"""Trainium2 Bass kernel for a top-2 gated MoE layer (8 experts, H=1024, F=4096).

Strategy (expert parallelism across the 8 NeuronCores):
  - Host computes the top-2 routing (argsort of the fp32 gate logits) and
    gathers each expert's tokens into a padded, transposed activation block
    xgT [H, C] (C = padded per-expert capacity).  All heavy math runs on
    device; the host only shards/gathers.
  - Each core runs one expert: gate logits + top-2 softmax weights are
    recomputed on device from its gathered tokens, LayerNorm + fc1 + gelu +
    fc2 + bias + gate scaling all happen on device (matmuls in bf16 with
    fp32 PSUM accumulation, LN statistics/scalars in fp32).
  - Host scatter-adds the per-expert outputs back into the full [B,S,H]
    tensor.

Self-contained: shapes are hardcoded from the problem spec.
"""

import numpy as np
import ml_dtypes
from contextlib import ExitStack

TOP_K = 2
LN_EPS = 1e-5
B, S, H, E, F = 2, 2048, 1024, 8, 4096
T = B * S
P = 128
KH = H // P          # 8 H-tiles
FB = 1024            # F block size
NFB = F // FB        # 4 blocks
MF = FB // P         # 8 F-tiles per block

_BUILD_CACHE = {}


def _chunks(C):
    # Small first chunk so the LN -> fc1 pipeline fills quickly.
    out = []
    off = 0
    if C >= 768:
        out.append((0, 256))
        off = 256
    while C - off > 512:
        out.append((off, 512))
        off += 512
    if C - off:
        out.append((off, C - off))
    return out


def _build(C):
    """Build + compile the single-core Bass program (SPMD across 8 cores)."""
    if C in _BUILD_CACHE:
        return _BUILD_CACHE[C]

    import concourse.bass as bass  # noqa: F401
    import concourse.tile as tile
    import concourse.mybir as mybir
    from concourse import bacc, bass_isa

    bf = mybir.dt.bfloat16
    f32 = mybir.dt.float32
    AF = mybir.ActivationFunctionType
    OP = mybir.AluOpType

    nc = bacc.Bacc("TRN2", target_bir_lowering=False, debug=False, num_devices=8)

    d_xgT = nc.dram_tensor("xgT", [H, C], bf, kind="ExternalInput")
    d_w1 = nc.dram_tensor("w1", [H, F], bf, kind="ExternalInput")
    d_w2 = nc.dram_tensor("w2", [F, H], bf, kind="ExternalInput")
    d_wgr = nc.dram_tensor("wgr", [P, KH, 32 + E], bf, kind="ExternalInput")
    d_b1r = nc.dram_tensor("b1r", [P, F // P], f32, kind="ExternalInput")
    d_pp = nc.dram_tensor("pp", [P, 3 * KH], f32, kind="ExternalInput")
    d_hea = nc.dram_tensor("hea", [E, 2], f32, kind="ExternalInput")
    d_y = nc.dram_tensor("ytT", [H, C], f32, kind="ExternalOutput")

    chunks = _chunks(C)

    with tile.TileContext(nc) as tc, ExitStack() as ctx:
        const = ctx.enter_context(tc.tile_pool(name="const", bufs=1))
        gpool = ctx.enter_context(tc.tile_pool(name="gate", bufs=1))
        bpool = ctx.enter_context(tc.tile_pool(name="bcast", bufs=1))
        xpool = ctx.enter_context(tc.tile_pool(name="x", bufs=1))
        sqpool = ctx.enter_context(tc.tile_pool(name="sq", bufs=8))
        tpool = ctx.enter_context(tc.tile_pool(name="t1", bufs=3))
        hpool = ctx.enter_context(tc.tile_pool(name="hdn", bufs=KH))
        w1pool = ctx.enter_context(tc.tile_pool(name="w1", bufs=2))
        w2pool = ctx.enter_context(tc.tile_pool(name="w2", bufs=1))
        apool = ctx.enter_context(tc.tile_pool(name="acts", bufs=8))
        ypool = ctx.enter_context(tc.tile_pool(name="yacc", bufs=1))
        ps_small = ctx.enter_context(
            tc.tile_pool(name="ps_small", bufs=2, space="PSUM"))
        ps1 = ctx.enter_context(tc.tile_pool(name="ps1", bufs=3, space="PSUM"))
        ps2 = ctx.enter_context(tc.tile_pool(name="ps2", bufs=3, space="PSUM"))

        # ---- constants / small params ----
        ones_k = const.tile([P, 1], bf)
        nc.vector.memset(ones_k, 1.0)
        # PE warm-up: ~5us of junk matmuls trains the HAM clock gate to
        # 2.4 GHz while the first x DMAs are still in flight.
        ones_m = const.tile([1, P], f32)
        nc.vector.memset(ones_m, 1.0)
        glib0 = const.tile([E, 1], f32)
        glib1 = const.tile([E, 1], f32)
        glibs = const.tile([E, 1], f32)
        glibr = const.tile([E, 1], f32)
        nc.vector.memset(glib0, 1.0)
        # pre-warm: gpsimd partition-op library, the Sqrt ACT table, and the
        # custom-DVE reciprocal uops — all during the initial x DMA wait, so
        # none of these one-time costs land on chunk 0's critical path
        nc.gpsimd.partition_all_reduce(glib1[:], glib0[:], E,
                                       bass_isa.ReduceOp.max)
        nc.scalar.activation(glibs[:], glib0[:], AF.Sqrt)
        nc.vector.reciprocal_approx_fast(out=glibr[:], in_=glib0[:])
        warm_rhs = const.tile([P, 512], bf)
        nc.vector.memset(warm_rhs, 0.0)
        ps_w = ps_small.tile([1, 512], f32, tag="pss", name="warm")
        for i in range(8):
            nc.tensor.matmul(ps_w[:], ones_k[:], warm_rhs[:],
                             start=True, stop=True)
        xbig = xpool.tile([P, KH, C], bf, tag="xk", name="xbig")
        xk = [xbig[:, k, :] for k in range(KH)]
        d_xr = d_xgT.ap().rearrange("(k p) c -> p k c", p=P)
        nc.sync.dma_start(xbig[:, :, 0:chunks[0][1]],
                          d_xr[:, :, 0:chunks[0][1]])
        wg_sb = const.tile([P, KH, 32 + E], bf)
        nc.sync.dma_start(wg_sb[:], d_wgr.ap())
        pp_sb = const.tile([P, 3 * KH], f32)
        nc.sync.dma_start(pp_sb[:], d_pp.ap())
        lnw_sb = pp_sb[:, 0:KH]
        lnb_sb = pp_sb[:, KH:2 * KH]
        b2_sb = pp_sb[:, 2 * KH:3 * KH]
        hea_sb = const.tile([E, 2], f32)
        nc.sync.dma_start(hea_sb[:], d_hea.ap())
        he_sb = hea_sb[:, 0:1]
        al_sb = hea_sb[:, 1:2]
        b1_sb = const.tile([P, F // P], f32)
        nc.sync.dma_start(b1_sb[:], d_b1r.ap())

        # ---- Phases A-C, pipelined along C-chunks so the PE can start the
        # fc1 matmuls of chunk 0 while later chunks are still in LN/gate ----
        eps_t = gpool.tile([1, 1], f32)
        nc.vector.memset(eps_t, float(LN_EPS))
        for (off, w) in chunks[1:]:
            nc.sync.dma_start(xbig[:, :, off:off + w], d_xr[:, :, off:off + w])
        hdn = [hpool.tile([P, C], bf, tag="hdn", name=f"hdn{k}")
               for k in range(KH)]
        sums = gpool.tile([1, C], f32)
        sumsq = gpool.tile([1, C], f32)
        varb = gpool.tile([1, C], f32)
        l_sb = gpool.tile([E, C], f32)
        m1b = gpool.tile([E, C], f32)
        eqm = gpool.tile([E, C], f32)
        comb_row = gpool.tile([1, C], f32)
        sums_b = bpool.tile([P, C], f32)
        inv_b = bpool.tile([P, C], f32)
        comb_b = bpool.tile([P, C], f32)

        sq_t = {}
        for ci, (off, w) in enumerate(chunks):
            for k in range(KH):
                sq_c = sqpool.tile([P, w], bf, tag="sq", name=f"sq_{off}_{k}")
                nc.vector.tensor_mul(sq_c[:], xk[k][:, off:off + w],
                                     xk[k][:, off:off + w])
                sq_t[(ci, k)] = sq_c
        ybig = ypool.tile([P, KH, C], f32, tag="yacc", name="ybig")
        y_acc = [ybig[:, h, :] for h in range(KH)]
        d_yr = d_y.ap().rearrange("(k p) c -> p k c", p=P)

        def load_w_block(fb):
            w1blk = w1pool.tile([P, KH, FB], bf, tag="w1", name=f"w1_{fb}")
            nc.sync.dma_start(
                w1blk[:],
                d_w1.ap()[:, fb * FB:(fb + 1) * FB].rearrange(
                    "(k p) f -> p k f", p=P))
            w2blk = w2pool.tile([P, MF, H], bf, tag="w2", name=f"w2_{fb}")
            nc.sync.dma_start(
                w2blk[:],
                d_w2.ap()[fb * FB:(fb + 1) * FB, :].rearrange(
                    "(k p) h -> p k h", p=P))
            return ([w1blk[:, k, :] for k in range(KH)],
                    [w2blk[:, k, :] for k in range(MF)])

        w1t0, w2t0 = load_w_block(0)
        at0 = [apool.tile([P, C], bf, tag="acts", name=f"a_0_{m}")
               for m in range(MF)]

        def emit_prologue(ci):
            off, w = chunks[ci]
            sl = slice(off, off + w)
            # column sums + gate logits in ONE PE reduction pass (lhsT is
            # the augmented [ones | Wg] matrix), sums of squares separately.
            ps_a = ps_small.tile([32 + E, w], f32, tag="pss",
                                 name=f"ps_sl{off}")
            for k in range(KH):
                nc.tensor.matmul(ps_a[:], wg_sb[:, k, :], xk[k][:, sl],
                                 start=(k == 0), stop=(k == KH - 1))
            nc.vector.tensor_copy(sums[:, sl], ps_a[0:1, :])
            nc.vector.tensor_copy(l_sb[:, sl], ps_a[32:32 + E, :])
            ps_b = ps_small.tile([1, w], f32, tag="pss", name=f"ps_sq{off}")
            for k in range(KH):
                nc.tensor.matmul(ps_b[:], ones_k[:], sq_t[(ci, k)][:],
                                 start=(k == 0), stop=(k == KH - 1))
            nc.vector.tensor_copy(sumsq[:, sl], ps_b[:])

            # LN stats; mean stays unnormalized (sums), 1/H is folded into
            # the apply step.  var = (sumsq - sums^2/H)/H via ACT scale.
            nc.vector.scalar_tensor_tensor(varb[:, sl], sums[:, sl], 1.0 / H,
                                           sums[:, sl], OP.mult, OP.mult)
            nc.vector.tensor_sub(varb[:, sl], sumsq[:, sl], varb[:, sl])
            nc.scalar.activation(sumsq[:, sl], varb[:, sl], AF.Sqrt,
                                 bias=eps_t[:], scale=1.0 / H)
            nc.vector.reciprocal_approx_fast(out=varb[:, sl],
                                             in_=sumsq[:, sl])
            nc.gpsimd.partition_broadcast(sums_b[:, sl], sums[0:1, sl], P)
            nc.gpsimd.partition_broadcast(inv_b[:, sl], varb[0:1, sl], P)

            # apply LayerNorm -> hdn (bf16):
            #   t1 = (sums_b/H - x) * lnw * inv ;  hdn = -t1 + lnb
            for k in range(KH):
                t1 = tpool.tile([P, w], f32, tag="t1", name=f"t1_{off}_{k}")
                nc.vector.scalar_tensor_tensor(t1[:], sums_b[:, sl], 1.0 / H,
                                               xk[k][:, sl],
                                               OP.mult, OP.subtract)
                nc.vector.scalar_tensor_tensor(t1[:], t1[:], lnw_sb[:, k:k + 1],
                                               inv_b[:, sl], OP.mult, OP.mult)
                nc.scalar.activation(hdn[k][:, sl], t1[:], AF.Identity,
                                     bias=lnb_sb[:, k:k + 1], scale=-1.0)

        def emit_fb0(ci):
            # F-block 0 fc1 -> gelu -> fc2 on this chunk
            off, w = chunks[ci]
            sl = slice(off, off + w)
            for m in range(MF):
                pst = ps1.tile([P, w], f32, tag="ps1", name=f"ps1_0_{m}_{ci}")
                for k in range(KH):
                    nc.tensor.matmul(pst[:], w1t0[k][:, m * P:(m + 1) * P],
                                     hdn[k][:, sl],
                                     start=(k == 0), stop=(k == KH - 1))
                nc.scalar.activation(at0[m][:, sl], pst[:],
                                     AF.Gelu_apprx_tanh,
                                     bias=b1_sb[:, m:m + 1])
            for h in range(KH):
                pst = ps2.tile([P, w], f32, tag="ps2", name=f"ps2_0_{h}_{ci}")
                for k in range(MF):
                    nc.tensor.matmul(pst[:], w2t0[k][:, h * P:(h + 1) * P],
                                     at0[k][:, sl],
                                     start=(k == 0), stop=(k == MF - 1))
                nc.scalar.activation(y_acc[h][:, sl], pst[:], AF.Identity,
                                     bias=0.0)

        # software pipeline: each chunk's stats/LN are emitted one chunk
        # ahead of its block-0 compute, so no engine's (in-order) queue makes
        # chunk c+1's LN wait behind chunk c's fb0-related work.
        emit_prologue(0)
        for ci in range(1, len(chunks)):
            emit_prologue(ci)
            emit_fb0(ci - 1)
        emit_fb0(len(chunks) - 1)


        def emit_gate():
            # top-2 gate over the full token range (single Sigmoid table
            # load, runs under block-1 PE work): for l_e in the top-2 set
            # the softmax weight is exactly sigmoid(2*l_e - m1 - m2).
            nc.gpsimd.partition_all_reduce(m1b[:], l_sb[:], E,
                                           bass_isa.ReduceOp.max)
            nc.vector.tensor_tensor(eqm[:], l_sb[:], m1b[:], OP.is_equal)
            nc.vector.scalar_tensor_tensor(eqm[:], eqm[:], -1e30, l_sb[:],
                                           OP.mult, OP.add)
            nc.gpsimd.partition_all_reduce(eqm[:], eqm[:], E,
                                           bass_isa.ReduceOp.max)  # m2
            nc.vector.tensor_add(m1b[:], m1b[:], eqm[:])  # m1 + m2
            nc.vector.scalar_tensor_tensor(l_sb[:], l_sb[:], 2.0, m1b[:],
                                           OP.mult, OP.subtract)
            nc.scalar.activation(l_sb[:], l_sb[:], AF.Sigmoid)
            nc.vector.tensor_scalar_mul(l_sb[:], l_sb[:], al_sb)
            for (off2, w2_) in chunks:
                ps_c = ps_small.tile([1, w2_], f32, tag="pss",
                                     name=f"ps_cmb{off2}")
                nc.tensor.matmul(ps_c[:], he_sb, l_sb[:, off2:off2 + w2_],
                                 start=True, stop=True)
                nc.vector.tensor_copy(comb_row[:, off2:off2 + w2_], ps_c[:])
            nc.gpsimd.partition_broadcast(comb_b[:], comb_row[:], P)

        # ---- Phase D: remaining F blocks.  Middle blocks iterate
        # weight-stationary (each lhsT feeds all chunks); the last block
        # iterates per-chunk so the finalize tail is short. ----
        for fb in range(1, NFB):
            w1t, w2t = load_w_block(fb)
            if fb == 2:
                emit_gate()

            at = [apool.tile([P, C], bf, tag="acts", name=f"a_{fb}_{m}")
                  for m in range(MF)]
            if fb == NFB - 1:
                order = sorted(range(len(chunks)),
                               key=lambda c: -chunks[c][1])
                ci_groups = [[ci] for ci in order]
            else:
                ci_groups = [list(range(len(chunks)))]

            for cig in ci_groups:
                for m in range(MF):
                    psg = {ci: ps1.tile([P, chunks[ci][1]], f32, tag="ps1",
                                        name=f"ps1_{fb}_{m}_{ci}")
                           for ci in cig}
                    for k in range(KH):
                        lhsT = w1t[k][:, m * P:(m + 1) * P]
                        for ci in cig:
                            off, w = chunks[ci]
                            nc.tensor.matmul(psg[ci][:], lhsT,
                                             hdn[k][:, off:off + w],
                                             start=(k == 0), stop=(k == KH - 1))
                    fcol = fb * MF + m
                    for ci in cig:
                        off, w = chunks[ci]
                        nc.scalar.activation(at[m][:, off:off + w], psg[ci][:],
                                             AF.Gelu_apprx_tanh,
                                             bias=b1_sb[:, fcol:fcol + 1])
                for h in range(KH):
                    psg = {ci: ps2.tile([P, chunks[ci][1]], f32, tag="ps2",
                                        name=f"ps2_{fb}_{h}_{ci}")
                           for ci in cig}
                    for k in range(MF):
                        lhsT = w2t[k][:, h * P:(h + 1) * P]
                        for ci in cig:
                            off, w = chunks[ci]
                            nc.tensor.matmul(psg[ci][:], lhsT,
                                             at[k][:, off:off + w],
                                             start=(k == 0), stop=(k == MF - 1))
                    for ci in cig:
                        off, w = chunks[ci]
                        if fb < NFB - 1:
                            nc.vector.tensor_add(y_acc[h][:, off:off + w],
                                                 y_acc[h][:, off:off + w],
                                                 psg[ci][:])
                        else:
                            # fused finalize: y = (psum + b2) + y_acc, then
                            # scale by the gate weight and store this chunk
                            nc.vector.scalar_tensor_tensor(
                                y_acc[h][:, off:off + w], psg[ci][:],
                                b2_sb[:, h:h + 1], y_acc[h][:, off:off + w],
                                OP.add, OP.add)
                            nc.vector.tensor_mul(y_acc[h][:, off:off + w],
                                                 y_acc[h][:, off:off + w],
                                                 comb_b[:, off:off + w])
                            nc.sync.dma_start(
                                d_yr[:, h:h + 1, off:off + w],
                                ybig[:, h:h + 1, off:off + w])

    nc.compile()
    _BUILD_CACHE[C] = nc
    return nc


def _prepare(x, Wg, alpha, ln_w, ln_b, fc1_w, fc1_b, fc2_w, fc2_b):
    """Host-side routing + per-core input construction."""
    bfnp = ml_dtypes.bfloat16
    xf = np.asarray(x, np.float32).reshape(T, H)
    Wg = np.asarray(Wg, np.float32)
    alpha = np.asarray(alpha, np.float32)
    ln_w = np.asarray(ln_w, np.float32)
    ln_b = np.asarray(ln_b, np.float32)
    fc1_w = np.asarray(fc1_w, np.float32)
    fc1_b = np.asarray(fc1_b, np.float32)
    fc2_w = np.asarray(fc2_w, np.float32)
    fc2_b = np.asarray(fc2_b, np.float32)

    logits = xf @ Wg
    order = np.argsort(-logits, axis=1, kind="stable")
    top2 = order[:, :TOP_K]
    sel = np.zeros((T, E), dtype=bool)
    sel[np.arange(T)[:, None], top2] = True
    idx = [np.nonzero(sel[:, e])[0] for e in range(E)]

    maxc = max(len(i) for i in idx)
    C = max(512, 16 * ((maxc + 15) // 16))

    KHp = H // 128
    wga = np.concatenate([np.ones((H, 1), np.float32),
                          np.zeros((H, 31), np.float32), Wg], axis=1)
    wgr = np.ascontiguousarray(
        wga.reshape(KHp, 128, 32 + E).transpose(1, 0, 2)).astype(bfnp)
    eye = np.eye(E, dtype=np.float32)
    in_maps = []
    for e in range(E):
        n = len(idx[e])
        xg = np.zeros((C, H), np.float32)
        xg[:n] = xf[idx[e]]
        pp = np.concatenate([
            ln_w[e].reshape(KHp, 128).T,
            ln_b[e].reshape(KHp, 128).T,
            fc2_b[e].reshape(KHp, 128).T,
        ], axis=1)
        hea = np.concatenate(
            [eye[:, e:e + 1], np.full((E, 1), alpha[e], np.float32)], axis=1)
        in_maps.append({
            "xgT": np.ascontiguousarray(xg.T).astype(bfnp),
            "w1": fc1_w[e].astype(bfnp),
            "w2": fc2_w[e].astype(bfnp),
            "wgr": wgr,
            "b1r": np.ascontiguousarray(fc1_b[e].reshape(F // 128, 128).T),
            "pp": np.ascontiguousarray(pp),
            "hea": np.ascontiguousarray(hea),
        })
    return in_maps, idx, C


def _kernel_impl(inputs, trace=False, trace_cores=None):
    from concourse import bass_utils

    in_maps, idx, C = _prepare(**inputs)
    nc = _build(C)
    res = bass_utils.run_bass_kernel_spmd(
        nc, in_maps, core_ids=list(range(E)),
        trace=trace, trace_cores=trace_cores)

    out = np.zeros((T, H), np.float32)
    for e in range(E):
        yt = np.asarray(res.results[e]["ytT"], np.float32)  # [H, C]
        n = len(idx[e])
        out[idx[e]] += yt.T[:n]
    return out.reshape(B, S, H), res


def kernel(**inputs):
    out, _ = _kernel_impl(inputs)
    return out



# revision 3
# speedup vs baseline: 1.1198x; 1.1198x over previous
"""Trainium2 Bass kernel for a top-2 gated MoE layer (8 experts, H=1024, F=4096).

Strategy (expert parallelism, host routing/LN, balanced 2-slot sharding):
  - Host computes routing (top-2 of the fp32 gate logits), the softmax gate
    weights, and the per-token LayerNorm (incl. per-expert ln_w/ln_b apply).
  - The 8192 token-expert pairs are packed into 16 slots: 8 slots of width
    s1 and 8 of width s2 (one of each per core), each slot holding tokens of
    a single expert.  A small solver picks (s1, s2) so C = s1+s2 is minimal
    (big experts take two s1 slots, small ones two s2 slots, the rest one of
    each) -- C ends up ~1048 instead of max-expert-count padding (~1104).
  - Each core runs a pure fc1 -> gelu(tanh) -> fc2 pipeline over its two
    slots (each slot has its own expert weights), bf16 matmuls with fp32
    PSUM accumulation, streaming the weights in F-blocks of 512.
  - Host applies the gate weight + fc2 bias while scatter-adding the
    per-slot outputs back into the full [B,S,H] tensor.

Self-contained: shapes are hardcoded from the problem spec.
"""

import numpy as np
import ml_dtypes
from contextlib import ExitStack

TOP_K = 2
LN_EPS = 1e-5
B, S, H, E, F = 2, 2048, 1024, 8, 4096
T = B * S
P = 128
KH = H // P          # 8 H-tiles
FB = 512             # F block size (weight streaming granularity)
NFB = F // FB        # 8 blocks
MFB = FB // P        # 4 F-tiles per block

_BUILD_CACHE = {}


def _seg_chunks(s):
    """Split a segment of s columns into PSUM-sized (<=512) chunks."""
    if s <= 512:
        return [(0, s)]
    c1 = ((s + 1) // 2 + 3) // 4 * 4
    return [(0, c1), (c1, s - c1)]


def _build(s1, s2):
    """Build + compile the single-core Bass program (SPMD across 8 cores)."""
    key = (s1, s2)
    if key in _BUILD_CACHE:
        return _BUILD_CACHE[key]

    import concourse.bass as bass  # noqa: F401
    import concourse.tile as tile
    import concourse.mybir as mybir
    from concourse import bacc

    bf = mybir.dt.bfloat16
    f32 = mybir.dt.float32
    AF = mybir.ActivationFunctionType

    C = s1 + s2
    segs = [(0, s1), (s1, s2)]  # (column offset, width) per slot

    nc = bacc.Bacc("TRN2", target_bir_lowering=False, debug=False,
                   num_devices=8)

    d_x = nc.dram_tensor("xnT", [P, KH, C], bf, kind="ExternalInput")
    d_w1 = [nc.dram_tensor(f"w1{s}", [NFB, P, KH, FB], bf,
                           kind="ExternalInput") for s in range(2)]
    d_w2 = [nc.dram_tensor(f"w2{s}", [NFB, P, MFB, H], bf,
                           kind="ExternalInput") for s in range(2)]
    d_b1 = nc.dram_tensor("b1r", [P, 2 * (F // P)], f32, kind="ExternalInput")
    d_y = nc.dram_tensor("ytT", [P, KH, C], f32, kind="ExternalOutput")

    with tile.TileContext(nc) as tc, ExitStack() as ctx:
        const = ctx.enter_context(tc.tile_pool(name="const", bufs=1))
        xpool = ctx.enter_context(tc.tile_pool(name="x", bufs=1))
        w1pool = ctx.enter_context(tc.tile_pool(name="w1", bufs=4))
        w2pool = ctx.enter_context(tc.tile_pool(name="w2", bufs=4))
        apool = ctx.enter_context(tc.tile_pool(name="acts", bufs=2))
        ypool = ctx.enter_context(tc.tile_pool(name="yacc", bufs=1))
        ps1 = ctx.enter_context(tc.tile_pool(name="ps1", bufs=4, space="PSUM"))
        ps2 = ctx.enter_context(tc.tile_pool(name="ps2", bufs=4, space="PSUM"))

        # ---- warmup: ~3us of junk matmuls trains the PE clock gate to full
        # speed while the first x/w DMAs are in flight; also pre-load the
        # Gelu ACT table so the first real gelu doesn't stall on it. ----
        ones_k = const.tile([P, 1], bf)
        nc.vector.memset(ones_k, 1.0)
        warm_rhs = const.tile([P, 512], bf)
        nc.vector.memset(warm_rhs, 0.0)
        gtab = const.tile([P, 1], f32)
        nc.scalar.activation(gtab[:], ones_k[:], AF.Gelu_apprx_tanh)
        ps_w = ps1.tile([1, 512], f32, tag="ps1", name="warm")
        for i in range(8):
            nc.tensor.matmul(ps_w[:], ones_k[:], warm_rhs[:],
                             start=True, stop=True)

        # ---- input DMAs: first chunk of x first, then first w1 block by
        # m-tile (so the first fc1 group can start ASAP), then the rest. ----
        xsb = xpool.tile([P, KH, C], bf, tag="x", name="xsb")
        ck0 = _seg_chunks(s1)[0][1]
        nc.sync.dma_start(xsb[:, :, 0:ck0], d_x.ap()[:, :, 0:ck0])

        def load_w(seg, fb, split_m=False):
            w1t = w1pool.tile([P, KH, FB], bf, tag="w1", name=f"w1_{seg}_{fb}")
            if split_m:
                for m in range(MFB):
                    nc.sync.dma_start(
                        w1t[:, :, m * P:(m + 1) * P],
                        d_w1[seg].ap()[fb][:, :, m * P:(m + 1) * P])
            else:
                nc.sync.dma_start(w1t[:], d_w1[seg].ap()[fb])
            w2t = w2pool.tile([P, MFB, H], bf, tag="w2", name=f"w2_{seg}_{fb}")
            nc.sync.dma_start(w2t[:], d_w2[seg].ap()[fb])
            return w1t, w2t

        wt = {}
        wt[(0, 0)] = load_w(0, 0, split_m=True)
        nc.sync.dma_start(xsb[:, :, ck0:C], d_x.ap()[:, :, ck0:C])
        wt[(1, 0)] = load_w(1, 0)

        b1sb = const.tile([P, 2 * (F // P)], f32)
        nc.sync.dma_start(b1sb[:], d_b1.ap())

        ysb = ypool.tile([P, KH, C], f32, tag="y", name="ysb")
        d_yr = d_y.ap()

        # ---- main pipeline: for each F-block, for each slot:
        # fc1 (all m-tiles) -> gelu -> fc2 (all h-tiles) -> y accumulate ----
        for fb in range(NFB):
            # prefetch next block's weights (both slots)
            if fb + 1 < NFB:
                wt[(0, fb + 1)] = load_w(0, fb + 1)
                wt[(1, fb + 1)] = load_w(1, fb + 1)
            for seg in range(2):
                soff, swid = segs[seg]
                w1t, w2t = wt.pop((seg, fb))
                chunks = _seg_chunks(swid)
                asb = apool.tile([P, MFB, swid], bf, tag="acts",
                                 name=f"a_{seg}_{fb}")
                for m in range(MFB):
                    for (off, w) in chunks:
                        pst = ps1.tile([P, w], f32, tag="ps1",
                                       name=f"ps1_{fb}_{seg}_{m}_{off}")
                        for k in range(KH):
                            nc.tensor.matmul(
                                pst[:], w1t[:, k, m * P:(m + 1) * P],
                                xsb[:, k, soff + off:soff + off + w],
                                start=(k == 0), stop=(k == KH - 1))
                        fcol = seg * (F // P) + fb * MFB + m
                        nc.scalar.activation(asb[:, m, off:off + w], pst[:],
                                             AF.Gelu_apprx_tanh,
                                             bias=b1sb[:, fcol:fcol + 1])
                for h in range(KH):
                    for (off, w) in chunks:
                        pst = ps2.tile([P, w], f32, tag="ps2",
                                       name=f"ps2_{fb}_{seg}_{h}_{off}")
                        for m in range(MFB):
                            nc.tensor.matmul(
                                pst[:], w2t[:, m, h * P:(h + 1) * P],
                                asb[:, m, off:off + w],
                                start=(m == 0), stop=(m == MFB - 1))
                        ysl = ysb[:, h, soff + off:soff + off + w]
                        if fb == 0:
                            nc.scalar.activation(ysl, pst[:], AF.Identity,
                                                 bias=0.0)
                        else:
                            nc.vector.tensor_add(ysl, ysl, pst[:])
                            if fb == NFB - 1:
                                nc.sync.dma_start(
                                    d_yr[:, h:h + 1,
                                         soff + off:soff + off + w],
                                    ysb[:, h:h + 1,
                                        soff + off:soff + off + w])

    nc.compile()
    _BUILD_CACHE[key] = nc
    return nc


def _plan(counts):
    """Pick slot widths (s1, s2) and assign experts to the 16 slots.

    Config family indexed by x: the x biggest experts take two s1-slots,
    the x smallest take two s2-slots, the middle 8-2x take one of each.
    Returns (s1, s2, s1_pieces, s2_pieces) where each piece is
    (expert_id, n_tokens, token_offset_within_expert).
    """
    order = np.argsort(-np.asarray(counts), kind="stable")
    cs = [int(counts[e]) for e in order]

    best = None
    for x in range(0, 5):
        if x == 0:
            s1 = (cs[0] + 1) // 2
            s2 = cs[0] - s1
        else:
            s1 = (cs[0] + 1) // 2
            s2 = (cs[8 - x] + 1) // 2 if x >= 1 else 0
            mid = cs[x:8 - x]
            if mid and mid[0] > s1 + s2:
                s2 = mid[0] - s1
        s1 = max(s1, s2)
        cval = s1 + s2
        if best is None or cval < best[0]:
            best = (cval, x, s1, s2)
    _, x, s1, s2 = best
    # round up to multiples of 4 for DMA alignment
    s1 = (s1 + 3) // 4 * 4
    s2 = (s2 + 3) // 4 * 4

    s1_pieces, s2_pieces = [], []
    for i, e in enumerate(order):
        c = cs[i]
        if i < x:                       # two s1 slots
            a = (c + 1) // 2
            s1_pieces += [(int(e), a, 0), (int(e), c - a, a)]
        elif i >= 8 - x:                # two s2 slots
            a = (c + 1) // 2
            s2_pieces += [(int(e), a, 0), (int(e), c - a, a)]
        else:                           # one of each
            a = min(c, s1)
            s1_pieces.append((int(e), a, 0))
            s2_pieces.append((int(e), c - a, a))
    assert len(s1_pieces) == 8 and len(s2_pieces) == 8
    for (_, n, _o) in s1_pieces:
        assert n <= s1
    for (_, n, _o) in s2_pieces:
        assert n <= s2
    return s1, s2, s1_pieces, s2_pieces


def _prepare(x, Wg, alpha, ln_w, ln_b, fc1_w, fc1_b, fc2_w, fc2_b):
    """Host-side routing + LN + per-core input construction."""
    bfnp = ml_dtypes.bfloat16
    xf = np.asarray(x, np.float32).reshape(T, H)
    Wg = np.asarray(Wg, np.float32)
    alpha = np.asarray(alpha, np.float32)
    ln_w = np.asarray(ln_w, np.float32)
    ln_b = np.asarray(ln_b, np.float32)
    fc1_w = np.asarray(fc1_w, np.float32)
    fc1_b = np.asarray(fc1_b, np.float32)
    fc2_w = np.asarray(fc2_w, np.float32)
    fc2_b = np.asarray(fc2_b, np.float32)

    # routing (matches jax.lax.top_k tie-breaking: lowest index wins)
    logits = xf @ Wg
    order = np.argsort(-logits, axis=1, kind="stable")
    top2 = order[:, :TOP_K]
    tv = np.take_along_axis(logits, top2, axis=1)
    tv = tv - tv.max(axis=1, keepdims=True)
    ev = np.exp(tv)
    gsc = ev / ev.sum(axis=1, keepdims=True)          # [T, 2] softmax
    idx = [None] * E
    gw = [None] * E
    for e in range(E):
        sel = top2 == e                               # [T, 2]
        rows = np.nonzero(sel.any(axis=1))[0]
        idx[e] = rows
        gw[e] = gsc[rows][sel[rows]] * alpha[e]
    counts = [len(i) for i in idx]

    s1, s2, s1_pieces, s2_pieces = _plan(counts)
    C = s1 + s2

    # per-token LN (stats in fp32), per-expert scale/shift applied at gather
    mu = xf.mean(axis=1, keepdims=True)
    xc = xf - mu
    var = np.square(xc).mean(axis=1, keepdims=True)
    xn = xc / np.sqrt(var + LN_EPS)                   # [T, H]

    # per-expert packed weights (shared across cores via the same arrays)
    w1r = {}
    w2r = {}
    b1r = {}
    for e in set(p[0] for p in s1_pieces + s2_pieces):
        w1r[e] = np.ascontiguousarray(
            fc1_w[e].reshape(KH, P, NFB, FB).transpose(2, 1, 0, 3)
        ).astype(bfnp)
        w2r[e] = np.ascontiguousarray(
            fc2_w[e].reshape(NFB, FB // P, P, H).transpose(0, 2, 1, 3)
        ).astype(bfnp)
        b1r[e] = np.ascontiguousarray(fc1_b[e].reshape(F // P, P).T)

    in_maps = []
    meta = []
    for core in range(E):
        pieces = [s1_pieces[core], s2_pieces[core]]
        xg = np.zeros((C, H), np.float32)
        offs = [0, s1]
        for (slot, (e, n, toff)) in enumerate(pieces):
            if n:
                rows = idx[e][toff:toff + n]
                xg[offs[slot]:offs[slot] + n] = \
                    xn[rows] * ln_w[e] + ln_b[e]
        xnT = np.ascontiguousarray(
            xg.reshape(C, KH, P).transpose(2, 1, 0)).astype(bfnp)
        b1c = np.concatenate([b1r[pieces[0][0]], b1r[pieces[1][0]]], axis=1)
        in_maps.append({
            "xnT": xnT,
            "w10": w1r[pieces[0][0]], "w11": w1r[pieces[1][0]],
            "w20": w2r[pieces[0][0]], "w21": w2r[pieces[1][0]],
            "b1r": np.ascontiguousarray(b1c),
        })
        meta.append(pieces)
    return in_maps, meta, idx, gw, fc2_b, s1, s2


def _kernel_impl(inputs, trace=False, trace_cores=None):
    from concourse import bass_utils

    in_maps, meta, idx, gw, fc2_b, s1, s2 = _prepare(**inputs)
    nc = _build(s1, s2)
    res = bass_utils.run_bass_kernel_spmd(
        nc, in_maps, core_ids=list(range(E)),
        trace=trace, trace_cores=trace_cores)

    C = s1 + s2
    out = np.zeros((T, H), np.float32)
    offs = [0, s1]
    for core in range(E):
        yt = np.asarray(res.results[core]["ytT"], np.float32)  # [P, KH, C]
        yflat = yt.transpose(2, 1, 0).reshape(C, H)            # [C, H]
        for (slot, (e, n, toff)) in enumerate(meta[core]):
            if n:
                rows = idx[e][toff:toff + n]
                w = gw[e][toff:toff + n][:, None]
                out[rows] += w * (yflat[offs[slot]:offs[slot] + n]
                                  + fc2_b[e])
    return out.reshape(B, S, H), res


def kernel(**inputs):
    out, _ = _kernel_impl(inputs)
    return out
